# revision 1
# baseline (speedup 1.0000x reference)
"""Trainium2 Bass kernel for MultiHeadLatentAttention.

Reference computation (B=2, S=2048, HIDDEN=2048, 16 heads x 128, LATENT=512):
  q_lat = x @ Wq_d ; kv_lat = x @ Wkv_d
  q = split_heads(q_lat @ Wq_u) ; k = split_heads(kv_lat @ Wk_u) ; v = split_heads(kv_lat @ Wv_u)
  q, k = rope(q, k)
  out = softmax(causal(q k^T / sqrt(d))) @ v   -> merge heads -> @ Wo

Sharding: 8 cores = 2 batches (data parallel) x 4-way tensor parallel over
heads (4 heads/core).  Each core computes the full latents for its batch
(replicated within the 4-core group), the up-projections + attention for its
4 heads, and a partial output projection over its heads' slice of Wo's input
dim.  The host sums the 4 partials per batch (cheap elementwise add).

Dataflow on-core is fully transposed ([feature, seq] layout) so no PE
transposes are needed anywhere:
  latT = Wd^T xT -> qT/kT per head via up-proj; rotate_half for rope is a
  single signed-permutation matmul on the PE; v in [seq, d] layout;
  scoresT[k, q] = kT-block-stationary x qT-moving; exp on ACT; softmax
  denominators via ones-vector matmuls accumulated on the PE; AV accumulated
  as v^T-stationary x expT; 1/denominator applied on the attention output
  (PSUM->SBUF copy fused); final Wo stage back in [seq, out] orientation.
  Causal structure skips above-diagonal blocks and narrows partial blocks.

Matmuls run in float32r (full PE rate; fp32 is 1/4 rate), fp32 accumulation.
"""

import sys
from contextlib import ExitStack

sys.path.insert(0, "/opt/trn_rl_repo")

import numpy as np

import concourse.bass as bass
import concourse.mybir as mybir
import concourse.tile as tile
from concourse import bacc
from concourse.bass_utils import run_bass_kernel_spmd

HIDDEN = 2048
LATENT = 512
NUM_HEADS = 16
HEAD_DIM = 128
THETA = 10000.0
B = 2
S_FULL = 2048
N_CORES = 8
TP = 4  # tensor-parallel group size (heads 16 / 4 = 4 per core)
HPC = NUM_HEADS // TP  # heads per core
DSL = HPC * HEAD_DIM  # per-core head-dim slice width (512)

F32 = mybir.dt.float32
F32R = mybir.dt.float32r

NEG = -1.0e30
SCALE = 1.0 / np.sqrt(HEAD_DIM)


def build_nc(S=S_FULL, finalize=True, iters=1, stages="full", variant="",
             gather=False):
    """Build the single-core SPMD program (same program all 8 cores).

    iters > 1 wraps the whole body in an on-device repeat loop (timing rig).
    """
    nc = bacc.Bacc(None, target_bir_lowering=False)

    KC_H = HIDDEN // 128   # 16 contraction chunks for hidden dim
    KC_L = LATENT // 128   # 4 contraction chunks for latent dim
    NB = S // 512          # number of 512-wide seq blocks
    SC = S // 128          # number of 128-wide seq chunks
    NH = S // 256          # number of 256-wide seq half-blocks (stage A)

    SQ = S // TP if gather else S
    NHL = SQ // 256 if gather else S // 256  # local half-blocks in stage A
    xT = nc.dram_tensor("xT", [HIDDEN, SQ], F32R, kind="ExternalInput")
    latpq = nc.dram_tensor("latpq", [128, LATENT // 128, SQ], F32R)
    latpkv = nc.dram_tensor("latpkv", [128, LATENT // 128, SQ], F32R)
    gq = nc.dram_tensor("gq", [TP, 128, LATENT // 128, SQ], F32R)
    gkv = nc.dram_tensor("gkv", [TP, 128, LATENT // 128, SQ], F32R)
    wqd = nc.dram_tensor("wqd", [HIDDEN, LATENT], F32R, kind="ExternalInput")
    wkvd = nc.dram_tensor("wkvd", [HIDDEN, LATENT], F32R, kind="ExternalInput")
    wqu = nc.dram_tensor("wqu", [LATENT, DSL], F32R, kind="ExternalInput")
    wku = nc.dram_tensor("wku", [LATENT, DSL], F32R, kind="ExternalInput")
    wvu = nc.dram_tensor("wvu", [LATENT, DSL], F32R, kind="ExternalInput")
    wo = nc.dram_tensor("wo", [DSL, HIDDEN], F32R, kind="ExternalInput")
    cosd = nc.dram_tensor("cosd", [128, S], F32, kind="ExternalInput")
    sind = nc.dram_tensor("sind", [128, S], F32, kind="ExternalInput")
    mtd = nc.dram_tensor("mtd", [2, 128, 256], F32, kind="ExternalInput")
    permd = nc.dram_tensor("permd", [128, 128], F32R, kind="ExternalInput")
    onesd = nc.dram_tensor("onesd", [128, 1], F32R, kind="ExternalInput")
    y = nc.dram_tensor("y", [S, HIDDEN], F32, kind="ExternalOutput")

    unroll = 1
    if iters < 0:          # negative: unrolled repeats (collective-safe)
        unroll, iters = -iters, 1
    with tile.TileContext(nc) as tc, ExitStack() as _es:
        if iters > 1:
            _es.enter_context(tc.For_i(0, iters, 1))
        for _u in range(unroll):
        # ---- persistent pools (allocated bottom of stack, live long) ----
          with tc.tile_pool(name="p_out", bufs=1) as p_out, \
               tc.tile_pool(name="p_lat", bufs=1) as p_lat, \
               tc.tile_pool(name="p_const", bufs=1) as p_const:

              outT = p_out.tile([128, HPC, S], F32R)      # attention out, transposed
              latq = p_lat.tile([128, KC_L, S], F32R)     # q_latT
              latkv = p_lat.tile([128, KC_L, S], F32R)    # kv_latT
              mask_sb = p_const.tile([128, 2, 256], F32)
              ones_sb = p_const.tile([128, 1], F32R)
              perm_sb = p_const.tile([128, 128], F32R)

              # ================= stage A: down projections =================
              with tc.tile_pool(name="p_wd", bufs=1) as p_wd, \
                   tc.tile_pool(name="p_xt", bufs=2) as p_xt, \
                   tc.tile_pool(name="p_st", bufs=4) as p_st, \
                   tc.tile_pool(name="ps_a", bufs=4, space="PSUM") as ps_a:
                  wqd_sb = p_wd.tile([128, KC_H, LATENT], F32R)
                  wkvd_sb = p_wd.tile([128, KC_H, LATENT], F32R)

                  def load_w_col(w_sb, w_dram, m):
                      nc.sync.dma_start(
                          out=w_sb[:, :, m * 128:(m + 1) * 128],
                          in_=w_dram.rearrange("(kc p) l -> p kc l", p=128)
                          [:, :, m * 128:(m + 1) * 128])

                  # prefetch order: wqd m=0 first, then slab 0 (in loop), then
                  # the rest, so the PE can start ~15us in.
                  load_w_col(wqd_sb, wqd, 0)
                  for nh in range(NHL):
                      xslab = p_xt.tile([128, KC_H, 256], F32R, tag="xslab")
                      nc.sync.dma_start(
                          out=xslab,
                          in_=xT.rearrange("(kc p) s -> p kc s", p=128)
                          [:, :, nh * 256:(nh + 1) * 256])
                      if nh == 0:
                          load_w_col(wkvd_sb, wkvd, 0)
                          for m in range(1, KC_L):
                              load_w_col(wqd_sb, wqd, m)
                              load_w_col(wkvd_sb, wkvd, m)
                          # constants (needed later; low priority)
                          nc.sync.dma_start(
                              out=mask_sb, in_=mtd.rearrange("j p c -> p j c"))
                          nc.sync.dma_start(out=ones_sb, in_=onesd[:, :])
                          nc.sync.dma_start(out=perm_sb, in_=permd[:, :])
                      _groups = []
                      if "X" not in variant:
                          for m in range(KC_L):
                              _groups.append((wqd_sb, latq, latpq, m))
                              _groups.append((wkvd_sb, latkv, latpkv, m))
                      for w_sb, lat, latp, m in _groups:
                          if True:
                              acc = ps_a.tile([128, 256], F32, tag="acc_a")
                              for kc in range(KC_H):
                                  nc.tensor.matmul(
                                      acc,
                                      w_sb[:, kc, m * 128:(m + 1) * 128],
                                      xslab[:, kc, :],
                                      start=(kc == 0), stop=(kc == KC_H - 1))
                              if gather:
                                  st = p_st.tile([128, 256], F32R, tag="st")
                                  nc.scalar.copy(st, acc)
                                  nc.sync.dma_start(
                                      out=latp[:, m, nh * 256:(nh + 1) * 256],
                                      in_=st)
                              else:
                                  nc.scalar.copy(
                                      lat[:, m, nh * 256:(nh + 1) * 256], acc)
                  if gather:
                      nc.gpsimd.collective_compute(
                          "AllGather", mybir.AluOpType.bypass,
                          replica_groups=[[0, 1, 2, 3], [4, 5, 6, 7]],
                          ins=[latpq[:, :, :]], outs=[gq[:, :, :, :]])
                      nc.gpsimd.collective_compute(
                          "AllGather", mybir.AluOpType.bypass,
                          replica_groups=[[0, 1, 2, 3], [4, 5, 6, 7]],
                          ins=[latpkv[:, :, :]], outs=[gkv[:, :, :, :]])
                      for m in range(KC_L):
                          nc.sync.dma_start(
                              out=latq[:, m, :],
                              in_=gq.rearrange("t p m s -> p m t s")[:, m, :, :])
                          nc.sync.dma_start(
                              out=latkv[:, m, :],
                              in_=gkv.rearrange("t p m s -> p m t s")[:, m, :, :])

              if stages == "a":
                  if "X" in variant:
                      nc.sync.dma_start(out=y[0:128, 0:LATENT],
                                        in_=wqd_sb[:, 0, :].bitcast(F32))
                      nc.sync.dma_start(out=y[128:256, 0:LATENT],
                                        in_=wkvd_sb[:, 0, :].bitcast(F32))
                  else:
                      nc.sync.dma_start(out=y[0:128, 0:S], in_=latq[:, 0, :].bitcast(F32))
                      nc.sync.dma_start(out=y[128:256, 0:S], in_=latkv[:, 0, :].bitcast(F32))
                      nc.sync.dma_start(out=y[256:384, 0:S], in_=latq[:, 2, :].bitcast(F32))
                      nc.sync.dma_start(out=y[384:512, 0:S], in_=latkv[:, 2, :].bitcast(F32))
              # ================= stage B0: v for all 4 heads ===============
              run_b = stages in ("av", "ac", "full")
              run_heads = stages in ("ac", "full")
              run_d = stages == "full"
              with tc.tile_pool(name="p_v", bufs=1) as p_v:
                  with tc.tile_pool(name="p_wv", bufs=1) as p_wv, \
                       tc.tile_pool(name="ps_v", bufs=4, space="PSUM") as ps_v:
                      v_sb = p_v.tile([128, SC, DSL], F32R)
                      wvu_sb = p_wv.tile([128, KC_L, DSL], F32R)
                      if run_b:
                          nc.sync.dma_start(
                              out=wvu_sb,
                              in_=wvu.rearrange("(kc p) d -> p kc d", p=128))
                      for sc in range(SC if run_b else 0):
                          acc = ps_v.tile([128, DSL], F32, tag="acc_v")
                          for kc in range(KC_L):
                              nc.tensor.matmul(
                                  acc,
                                  latkv[:, kc, sc * 128:(sc + 1) * 128],
                                  wvu_sb[:, kc, :],
                                  start=(kc == 0), stop=(kc == KC_L - 1))
                          nc.vector.tensor_copy(v_sb[:, sc, :], acc)

                  if stages == "av":
                      nc.sync.dma_start(out=y[0:128, 0:S], in_=latq[:, 0, :].bitcast(F32))
                      nc.sync.dma_start(out=y[128:256, 0:S], in_=latkv[:, 0, :].bitcast(F32))
                      nc.sync.dma_start(
                          out=y[256:384, 0:DSL],
                          in_=v_sb[:, SC - 1, :].bitcast(F32))
                  # ============ stages B/C per head: up-proj + attention ====
                  with tc.tile_pool(name="p_rope", bufs=1) as p_rope:
                      cos_sb = p_rope.tile([128, S], F32)
                      sin_sb = p_rope.tile([128, S], F32)
                      if run_heads:
                          nc.sync.dma_start(out=cos_sb, in_=cosd[:, :])
                          nc.sync.dma_start(out=sin_sb, in_=sind[:, :])
                      with tc.tile_pool(name="p_rt", bufs=3) as p_rt, \
                           tc.tile_pool(name="p_at", bufs=6) as p_at, \
                           tc.tile_pool(name="p_rb", bufs=2) as p_rb:
                        for h in range(HPC if run_heads else 0):
                          with tc.tile_pool(name="p_head", bufs=1) as p_head, \
                               tc.tile_pool(name="p_wu", bufs=2) as p_wu:
                              qT = p_head.tile([128, S], F32R, tag="qT")
                              kT = p_head.tile([128, S], F32R, tag="kT")
                              wq_sb = p_wu.tile([128, KC_L, 128], F32R, tag="wq")
                              wk_sb = p_wu.tile([128, KC_L, 128], F32R, tag="wk")
                              hs = h * 128
                              nc.sync.dma_start(
                                  out=wq_sb,
                                  in_=wqu.rearrange("(kc p) d -> p kc d", p=128)
                                  [:, :, hs:hs + 128])
                              nc.sync.dma_start(
                                  out=wk_sb,
                                  in_=wku.rearrange("(kc p) d -> p kc d", p=128)
                                  [:, :, hs:hs + 128])

                              with tc.tile_pool(name="ps_b", bufs=2,
                                                space="PSUM") as ps_b, \
                                   tc.tile_pool(name="ps_br", bufs=2,
                                                space="PSUM") as ps_br:
                                for dst, w_sb, lat in (
                                        (qT, wq_sb, latq),
                                        (kT, wk_sb, latkv)):
                                    for nb in range(NB):
                                        sl = slice(nb * 512, (nb + 1) * 512)
                                        pa = ps_b.tile([128, 512], F32, tag="pa")
                                        for kc in range(KC_L):
                                            nc.tensor.matmul(
                                                pa, w_sb[:, kc, :], lat[:, kc, sl],
                                                start=(kc == 0),
                                                stop=(kc == KC_L - 1))
                                        raw = p_rt.tile([128, 512], F32R, tag="raw")
                                        nc.vector.tensor_copy(raw, pa)
                                        pr = ps_br.tile([128, 512], F32, tag="pr")
                                        nc.tensor.matmul(pr, perm_sb, raw,
                                                         start=True, stop=True)
                                        rt = p_rt.tile([128, 512], F32, tag="rt")
                                        nc.vector.tensor_mul(dst[:, sl], pa,
                                                             cos_sb[:, sl])
                                        nc.vector.tensor_mul(rt, pr, sin_sb[:, sl])
                                        nc.vector.tensor_add(dst[:, sl],
                                                             dst[:, sl], rt)

                              # ---- attention for head h ----
                              with tc.tile_pool(name="ps_s", bufs=4,
                                                space="PSUM") as ps_s, \
                                   tc.tile_pool(name="ps_o", bufs=2,
                                                space="PSUM") as ps_o, \
                                   tc.tile_pool(name="ps_n", bufs=2,
                                                space="PSUM") as ps_n:
                                  for qb in range(NB):
                                      kb_hi = 4 * qb + 4
                                      po = ps_o.tile([128, 512], F32, tag="po")
                                      pn = ps_n.tile([1, 512], F32, tag="pn")
                                      for kb in range(kb_hi):
                                          j = kb - 4 * qb
                                          # narrowed q range for partial blocks
                                          # (keep N >= 256 for the f32r rate)
                                          off = min(j, 2) * 128 if j >= 0 else 0
                                          w = 512 - off
                                          q0 = qb * 512 + off
                                          ps = ps_s.tile([128, 512], F32, tag="ps")
                                          nc.tensor.matmul(
                                              ps[:, 0:w],
                                              kT[:, kb * 128:(kb + 1) * 128],
                                              qT[:, q0:q0 + w],
                                              start=True, stop=True)
                                          if j >= 0 and "M" not in variant:
                                              jj = j - off // 128
                                              mw = (jj + 1) * 128
                                              nc.vector.tensor_add(
                                                  ps[:, 0:mw], ps[:, 0:mw],
                                                  mask_sb[:, jj, 0:mw])
                                          et = p_at.tile([128, 512], F32R, tag="et")
                                          if "E" in variant:
                                              nc.vector.tensor_copy(et[:, 0:w],
                                                                    ps[:, 0:w])
                                          else:
                                              nc.scalar.activation(
                                                  out=et[:, 0:w], in_=ps[:, 0:w],
                                                  func=mybir.ActivationFunctionType.Exp,
                                                  scale=float(SCALE))
                                          nc.tensor.matmul(
                                              po[:, off:512],
                                              v_sb[:, kb, hs:hs + 128],
                                              et[:, 0:w],
                                              start=(kb == 0),
                                              stop=(kb == kb_hi - 1))
                                          if "O" not in variant:
                                              nc.tensor.matmul(
                                                  pn[0:1, off:512],
                                                  ones_sb[:, 0:1],
                                                  et[:, 0:w],
                                                  start=(kb == 0),
                                                  stop=(kb == kb_hi - 1))
                                      qsl = slice(qb * 512, (qb + 1) * 512)
                                      rc = p_rb.tile([1, 512], F32, tag="rc")
                                      if "O" in variant:
                                          nc.vector.reciprocal(rc, po[0:1, :])
                                      else:
                                          nc.vector.reciprocal(rc, pn[0:1, :])
                                      rb = p_rb.tile([128, 512], F32, tag="rb")
                                      nc.gpsimd.partition_broadcast(rb, rc)
                                      nc.vector.tensor_mul(outT[:, h, qsl], po, rb)

              if stages == "ac":
                  for h2 in range(HPC):
                      nc.sync.dma_start(out=y[h2 * 128:(h2 + 1) * 128, 0:S],
                                        in_=outT[:, h2, :].bitcast(F32))
              # ================= stage D: output projection ================
              with tc.tile_pool(name="p_wo", bufs=1) as p_wo, \
                   tc.tile_pool(name="p_fin", bufs=3) as p_fin, \
                   tc.tile_pool(name="ps_d", bufs=4, space="PSUM") as ps_d:
                  wo_sb = p_wo.tile([128, HPC, HIDDEN], F32R, tag="wo")
                  if run_d:
                      nc.sync.dma_start(
                          out=wo_sb, in_=wo.rearrange("(ic p) o -> p ic o", p=128))
                  for sc in range(SC if run_d else 0):
                      fin = p_fin.tile([128, HIDDEN], F32, tag="fin")
                      for ob in range(HIDDEN // 512):
                          acc = ps_d.tile([128, 512], F32, tag="acc_d")
                          for ic in range(HPC):
                              nc.tensor.matmul(
                                  acc,
                                  outT[:, ic, sc * 128:(sc + 1) * 128],
                                  wo_sb[:, ic, ob * 512:(ob + 1) * 512],
                                  start=(ic == 0), stop=(ic == HPC - 1))
                          osl = slice(ob * 512, (ob + 1) * 512)
                          if ob % 2 == 0:
                              nc.scalar.copy(fin[:, osl], acc)
                          else:
                              nc.vector.tensor_copy(fin[:, osl], acc)
                      nc.sync.dma_start(
                          out=y[sc * 128:(sc + 1) * 128, :], in_=fin)


    if finalize:
        nc.finalize()
    return nc


# ---------------------------------------------------------------------------
# host-side helpers


def host_inputs(x, Wq_d, Wkv_d, Wq_u, Wk_u, Wv_u, Wo, S=S_FULL, gather=False):
    """Build the 8 per-core input maps from full inputs."""
    x = np.asarray(x, dtype=np.float32)
    Wq_d = np.asarray(Wq_d, dtype=np.float32)
    Wkv_d = np.asarray(Wkv_d, dtype=np.float32)
    Wq_u = np.asarray(Wq_u, dtype=np.float32)
    Wk_u = np.asarray(Wk_u, dtype=np.float32)
    Wv_u = np.asarray(Wv_u, dtype=np.float32)
    Wo = np.asarray(Wo, dtype=np.float32)

    inv_freq = 1.0 / (THETA ** (np.arange(0, HEAD_DIM, 2, dtype=np.float64)
                                / HEAD_DIM))  # (64,)
    pos = np.arange(S, dtype=np.float64)
    ang = pos[None, :] * np.concatenate([inv_freq, inv_freq])[:, None]  # (128, S)
    COS = np.cos(ang).astype(np.float32)
    SIN = np.sin(ang).astype(np.float32)

    # masks for narrowed diagonal blocks: jj=0 -> mask c<r on first 128 cols;
    # jj=1 -> mask c<128+r on first 256 cols
    MT = np.zeros((2, 128, 256), dtype=np.float32)
    r = np.arange(128)[:, None]
    c = np.arange(256)[None, :]
    MT[0] = np.where(c >= r, 0.0, NEG)
    MT[1] = np.where(c >= 128 + r, 0.0, NEG)

    # signed permutation for rotate_half in [d, seq] layout:
    # out[m] = -in[m+64] for m<64 ; +in[m-64] for m>=64
    PERM = np.zeros((128, 128), dtype=np.float32)
    for m in range(64):
        PERM[m + 64, m] = -1.0
        PERM[m, m + 64] = 1.0

    in_maps = []
    for core in range(N_CORES):
        b, tp = core // TP, core % TP
        sl = slice(tp * DSL, (tp + 1) * DSL)
        SQ = S // TP if gather else S
        xt_c = x[b, :S].T
        if gather:
            xt_c = xt_c[:, tp * SQ:(tp + 1) * SQ]
        in_maps.append({
            "xT": np.ascontiguousarray(xt_c),
            "wqd": Wq_d,
            "wkvd": Wkv_d,
            "wqu": np.ascontiguousarray(Wq_u[:, sl]),
            "wku": np.ascontiguousarray(Wk_u[:, sl]),
            "wvu": np.ascontiguousarray(Wv_u[:, sl]),
            "wo": np.ascontiguousarray(Wo[sl, :]),
            "cosd": COS,
            "sind": SIN,
            "mtd": MT,
            "permd": PERM,
            "onesd": np.ones((128, 1), dtype=np.float32),
        })
    return in_maps


def assemble(results, S=S_FULL):
    out = np.zeros((B, S, HIDDEN), dtype=np.float32)
    for core in range(N_CORES):
        out[core // TP] += results[core]["y"]
    return out


_NC_CACHE = {}


def kernel(x, Wq_d, Wkv_d, Wq_u, Wk_u, Wv_u, Wo):
    S = x.shape[1]
    if S not in _NC_CACHE:
        _NC_CACHE[S] = build_nc(S)
    nc = _NC_CACHE[S]
    in_maps = host_inputs(x, Wq_d, Wkv_d, Wq_u, Wk_u, Wv_u, Wo, S=S)

    res = run_bass_kernel_spmd(nc, in_maps, list(range(N_CORES)))
    return assemble(res.results, S=S)



# revision 2
# speedup vs baseline: 2.0115x; 2.0115x over previous
"""Trainium2 Bass kernel for MultiHeadLatentAttention (v2.3).

Structural changes vs v1 (420us baseline):
 - Stage A (down-projections) sharded over the 4-core TP group by sequence
   (each core computes S/4=512 positions of both latents), then one AllGather
   (bf16, 1MB/rank) rebuilds the full latents.  Saves ~3/4 of the largest
   GEMM's replicated PE time; the collective runs on TOPSP/SDMA silicon.
 - bf16 everywhere on the data path (same PE rate as f32r, 2x less DMA/SBUF,
   faster DVE), fp32 accumulation in PSUM throughout.
 - RoPE via the tan identity: rope(pa) = cos o (pa + P @ (pa o tan)), with the
   signed-permutation matmul ACCUMULATED onto the up-projection's PSUM bank.
   2 DVE ops per 512-block instead of 4.
 - Causal masks added on the PE (identity-stationary matmul accumulate into
   the score bank) instead of DVE tensor_adds.
 - exp(scores) stays on ACT (the only engine with a LUT); its ~60-80us is the
   attention-phase floor alongside the PE.
 - Attention runs in 2 passes of 2 heads so PSUM fits:
   2 po + 2 pn + 4 score bufs = 8 banks.
Layout is fully transposed ([feature, seq]) as v1: no PE transposes anywhere.
"""

import sys
from contextlib import ExitStack

sys.path.insert(0, "/opt/trn_rl_repo")

import numpy as np

import concourse.bass as bass
import concourse.mybir as mybir
import concourse.tile as tile
from concourse import bacc
from concourse.bass_utils import run_bass_kernel_spmd

HIDDEN = 2048
LATENT = 512
NUM_HEADS = 16
HEAD_DIM = 128
THETA = 10000.0
B = 2
S_FULL = 2048
N_CORES = 8
TP = 4
HPC = NUM_HEADS // TP          # 4 heads per core
DSL = HPC * HEAD_DIM           # 512: per-core slice of head dims

F32 = mybir.dt.float32
BF = mybir.dt.bfloat16
F8 = mybir.dt.float8e4

NEG = -1.0e30
SCALE = 1.0 / np.sqrt(HEAD_DIM)


def build_nc(S=S_FULL, finalize=True, iters=1, variant="", fp8=""):
    nc = bacc.Bacc(None, target_bir_lowering=False)

    KC_H = HIDDEN // 128       # 16
    KC_L = LATENT // 128       # 4
    NB = S // 512              # 4
    SC = S // 128              # 16
    SQ = S // TP               # 512 local seq positions for stage A

    DT_A = F8 if "A" in fp8 else BF
    DT_D = F8 if "D" in fp8 else BF
    xT = nc.dram_tensor("xT", [HIDDEN, SQ], DT_A, kind="ExternalInput")
    wqd = nc.dram_tensor("wqd", [HIDDEN, LATENT], DT_A, kind="ExternalInput")
    wkvd = nc.dram_tensor("wkvd", [HIDDEN, LATENT], DT_A, kind="ExternalInput")
    wqu = nc.dram_tensor("wqu", [LATENT, DSL], BF, kind="ExternalInput")
    wku = nc.dram_tensor("wku", [LATENT, DSL], BF, kind="ExternalInput")
    wvu = nc.dram_tensor("wvu", [LATENT, DSL], BF, kind="ExternalInput")
    wo = nc.dram_tensor("wo", [DSL, HIDDEN], DT_D, kind="ExternalInput")
    cosd = nc.dram_tensor("cosd", [128, S], F32, kind="ExternalInput")
    tand = nc.dram_tensor("tand", [128, S], F32, kind="ExternalInput")
    mtd = nc.dram_tensor("mtd", [2, 128, 256], BF, kind="ExternalInput")
    identd = nc.dram_tensor("identd", [128, 128], BF, kind="ExternalInput")
    permd = nc.dram_tensor("permd", [128, 128], BF, kind="ExternalInput")
    onesd = nc.dram_tensor("onesd", [128, 1], BF, kind="ExternalInput")
    latp = [nc.dram_tensor("latpq", [128, KC_L, SQ], BF),
            nc.dram_tensor("latpkv", [128, KC_L, SQ], BF)]
    latg = [nc.dram_tensor("latgq", [TP, 128, KC_L, SQ], BF),
            nc.dram_tensor("latgkv", [TP, 128, KC_L, SQ], BF)]
    y = nc.dram_tensor("y", [S, HIDDEN], BF, kind="ExternalOutput")

    unroll = 1
    if iters < 0:              # unrolled repeats (collective-safe timing)
        unroll, iters = -iters, 1
    with tile.TileContext(nc) as tc, ExitStack() as _es:
        if iters > 1:
            raise ValueError("use negative iters (unroll); collective "
                             "cannot sit inside For_i")
        for _u in range(unroll):
          with tc.tile_pool(name="p_out", bufs=1) as p_out, \
               tc.tile_pool(name="p_lat", bufs=1) as p_lat, \
               tc.tile_pool(name="p_tab", bufs=1) as p_tab, \
               tc.tile_pool(name="p_wpre", bufs=1) as p_wpre, \
               tc.tile_pool(name="p_const", bufs=1) as p_const:

            outT = p_out.tile([128, HPC, S], DT_D)
            latq = p_lat.tile([128, KC_L, S], BF)
            latkv = p_lat.tile([128, KC_L, S], BF)
            cos_sb = p_tab.tile([128, S], F32)
            tan_sb = p_tab.tile([128, S], F32)
            wqku_sb = p_wpre.tile([128, KC_L, 2, DSL], BF)
            wvu_sb = p_wpre.tile([128, KC_L, DSL], BF)
            wo_sb = p_wpre.tile([128, HPC, HIDDEN], DT_D)
            mask_sb = p_const.tile([128, 2, 256], BF)
            ones_sb = p_const.tile([128, 1], BF)
            perm_sb = p_const.tile([128, 128], BF)
            ident_sb = p_const.tile([128, 128], BF)

            # ================= stage A: sharded down projections ==========
            with tc.tile_pool(name="p_wd", bufs=1) as p_wd, \
                 tc.tile_pool(name="p_xt", bufs=1) as p_xt, \
                 tc.tile_pool(name="p_st", bufs=4) as p_st, \
                 tc.tile_pool(name="ps_a", bufs=4, space="PSUM") as ps_a:
                wqd_sb = p_wd.tile([128, KC_H, LATENT], DT_A)
                wkvd_sb = p_wd.tile([128, KC_H, LATENT], DT_A)
                x_sb = p_xt.tile([128, KC_H, SQ], DT_A)

                def load_w_col(w_sb, w_dram, m):
                    nc.sync.dma_start(
                        out=w_sb[:, :, m * 128:(m + 1) * 128],
                        in_=w_dram.rearrange("(kc p) l -> p kc l", p=128)
                        [:, :, m * 128:(m + 1) * 128])

                # prefetch: wqd m=0, x, remaining wd cols, then the rest of
                # the kernel's weights/tables (they overlap stage A compute).
                load_w_col(wqd_sb, wqd, 0)
                xr = xT.rearrange("(kc p) s -> p kc s", p=128)
                for kq in range(4):
                    nc.sync.dma_start(
                        out=x_sb[:, 4 * kq:4 * kq + 4, :],
                        in_=xr[:, 4 * kq:4 * kq + 4, :])
                load_w_col(wkvd_sb, wkvd, 0)
                for m in range(1, KC_L):
                    load_w_col(wqd_sb, wqd, m)
                    load_w_col(wkvd_sb, wkvd, m)
                nc.sync.dma_start(out=mask_sb, in_=mtd.rearrange("j p c -> p j c"))
                nc.sync.dma_start(out=ones_sb, in_=onesd[:, :])
                nc.sync.dma_start(out=perm_sb, in_=permd[:, :])
                nc.sync.dma_start(out=ident_sb, in_=identd[:, :])
                nc.sync.dma_start(out=cos_sb, in_=cosd[:, :])
                nc.sync.dma_start(out=tan_sb, in_=tand[:, :])
                nc.sync.dma_start(
                    out=wqku_sb[:, :, 0, :],
                    in_=wqu.rearrange("(kc p) d -> p kc d", p=128))
                nc.sync.dma_start(
                    out=wqku_sb[:, :, 1, :],
                    in_=wku.rearrange("(kc p) d -> p kc d", p=128))
                nc.sync.dma_start(
                    out=wvu_sb, in_=wvu.rearrange("(kc p) d -> p kc d", p=128))
                nc.sync.dma_start(
                    out=wo_sb, in_=wo.rearrange("(ic p) o -> p ic o", p=128))

                # preload the exp table set while stage A computes
                warm = p_st.tile([1, 1], BF, tag="warm")
                nc.scalar.activation(
                    out=warm, in_=ones_sb[0:1, 0:1],
                    func=mybir.ActivationFunctionType.Exp, scale=1.0)

                if "X" not in variant:
                    # q latent groups first, gather them while the kv groups
                    # compute; then the kv gather hides under q up-proj.
                    for g in range(2 * KC_L):
                        w_sb = wqd_sb if g < KC_L else wkvd_sb
                        m = g % KC_L
                        acc = ps_a.tile([128, SQ], F32, tag="acc_a")
                        if "A" in fp8:
                            for kp in range(KC_H // 2):
                                nc.tensor.matmul(
                                    acc,
                                    w_sb[:, 2 * kp:2 * kp + 2,
                                         m * 128:(m + 1) * 128],
                                    x_sb[:, 2 * kp:2 * kp + 2, :],
                                    start=(kp == 0),
                                    stop=(kp == KC_H // 2 - 1),
                                    perf_mode=mybir.MatmulPerfMode.DoubleRow)
                        else:
                            for kc in range(KC_H):
                                nc.tensor.matmul(
                                    acc,
                                    w_sb[:, kc, m * 128:(m + 1) * 128],
                                    x_sb[:, kc, :],
                                    start=(kc == 0), stop=(kc == KC_H - 1))
                        st = p_st.tile([128, SQ], BF, tag="st")
                        nc.vector.tensor_copy(st, acc)
                        nc.sync.dma_start(
                            out=latp[g // KC_L][:, g % KC_L, :], in_=st)
                        if g == KC_L - 1:
                            _gather_lat(nc, latp[0], latg[0], latq, variant)
                    _gather_lat(nc, latp[1], latg[1], latkv, variant)

            if stages_done(variant, "a"):
                nc.sync.dma_start(out=y[0:128, 0:S],
                                  in_=latq[:, 0, :].bitcast(F32))
                nc.sync.dma_start(out=y[128:256, 0:S],
                                  in_=latkv[:, 0, :].bitcast(F32))

            run_b = "A" not in variant
            # ========== stages B0/B: v + up-proj/rope, order chosen so the
            # kv AllGather hides under the q up-projections ================
            with tc.tile_pool(name="p_v", bufs=1) as p_v:
                v_sb = p_v.tile([128, SC, DSL], BF)

                # ============= stage B: up-proj + rope, all heads =========
                with tc.tile_pool(name="p_qk", bufs=1) as p_qk:
                    qkT = p_qk.tile([128, HPC, 2, S], BF)
                    with tc.tile_pool(name="p_rt", bufs=3) as p_rt, \
                         tc.tile_pool(name="ps_v", bufs=3,
                                      space="PSUM") as ps_v, \
                         tc.tile_pool(name="ps_b", bufs=2,
                                      space="PSUM") as ps_b:

                        def up_rope(h, di):
                            # two 512-blocks share a 2-bank psum tile so the
                            # rope DVE ops run 1024 wide (amortized overhead)
                            lat = latq if di == 0 else latkv
                            for np2 in range(NB // 2):
                                sl = slice(np2 * 1024, (np2 + 1) * 1024)
                                pa = ps_b.tile([128, 2, 512], F32, tag="pa")
                                for half in range(2):
                                    hs = slice((2 * np2 + half) * 512,
                                               (2 * np2 + half + 1) * 512)
                                    for kc in range(KC_L):
                                        nc.tensor.matmul(
                                            pa[:, half, :],
                                            wqku_sb[:, kc, di,
                                                    h * 128:(h + 1) * 128],
                                            lat[:, kc, hs],
                                            start=(kc == 0),
                                            stop=(kc == KC_L - 1))
                                t = p_rt.tile([128, 2, 512], BF, tag="t")
                                nc.vector.tensor_mul(t, pa, tan_sb[:, sl])
                                for half in range(2):
                                    nc.tensor.matmul(pa[:, half, :], perm_sb,
                                                     t[:, half, :],
                                                     start=False, stop=True,
                                                     skip_group_check=True)
                                nc.vector.tensor_mul(qkT[:, h, di, sl],
                                                     pa, cos_sb[:, sl])

                        def v_proj(sc):
                            acc = ps_v.tile([128, DSL], F32, tag="acc_v")
                            for kc in range(KC_L):
                                nc.tensor.matmul(
                                    acc,
                                    latkv[:, kc, sc * 128:(sc + 1) * 128],
                                    wvu_sb[:, kc, :],
                                    start=(kc == 0), stop=(kc == KC_L - 1))
                            nc.vector.tensor_copy(v_sb[:, sc, :], acc)

                        if run_b:
                            for h in range(HPC):
                                up_rope(h, 0)          # q (hides kv gather)
                            up_rope(0, 1)
                            up_rope(1, 1)              # k for pass 0
                            for sc in range(SC):
                                v_proj(sc)
                            up_rope(2, 1)
                            up_rope(3, 1)              # k for pass 1

                    # ============= stage C: attention, 2 heads per pass ===
                    with tc.tile_pool(name="p_at", bufs=6) as p_at, \
                         tc.tile_pool(name="p_rb", bufs=4) as p_rb:
                      for pp in range(2 if run_b else 0):
                        hh = (2 * pp, 2 * pp + 1)
                        with tc.tile_pool(name="ps_s", bufs=3,
                                          space="PSUM") as ps_s, \
                             tc.tile_pool(name="ps_o", bufs=4,
                                          space="PSUM") as ps_o, \
                             tc.tile_pool(name="ps_n", bufs=1,
                                          space="PSUM") as ps_n:
                          for qb in range(NB):
                            kb_hi = 4 * qb + 4
                            po = {}
                            for h in hh:
                                po[h] = ps_o.tile([128, 512], F32, tag="po",
                                                  name=f"po{h}")
                            # one bank holds both heads' denominators, at
                            # partitions 0 and 32 (col-groups 0/1 of the PE
                            # array so the two ones-matmuls run concurrently)
                            pn = ps_n.tile([128, 512], F32, tag="pn")
                            for kb in range(kb_hi):
                                j = kb - 4 * qb
                                off = min(j, 2) * 128 if j >= 0 else 0
                                w = 512 - off
                                q0 = qb * 512 + off
                                ets = {}
                                for h in hh:
                                    masked = (j >= 0 and "M" not in variant)
                                    ps = ps_s.tile([128, 512], F32, tag="ps")
                                    nc.tensor.matmul(
                                        ps[:, 0:w],
                                        qkT[:, h, 1,
                                            kb * 128:(kb + 1) * 128],
                                        qkT[:, h, 0, q0:q0 + w],
                                        start=True, stop=not masked)
                                    if masked:
                                        jj = j - off // 128
                                        mw = (jj + 1) * 128
                                        nc.tensor.matmul(
                                            ps[:, 0:mw], ident_sb,
                                            mask_sb[:, jj, 0:mw],
                                            start=False, stop=True,
                                            skip_group_check=True)
                                    et = p_at.tile([128, 512], BF, tag="et",
                                                   name=f"et{h}")
                                    if "E" in variant:
                                        nc.vector.tensor_copy(et[:, 0:w],
                                                              ps[:, 0:w])
                                    else:
                                        nc.scalar.activation(
                                            out=et[:, 0:w], in_=ps[:, 0:w],
                                            func=mybir.ActivationFunctionType.Exp,
                                            scale=float(SCALE))
                                    ets[h] = et
                                    nc.tensor.matmul(
                                        po[h][:, off:512],
                                        v_sb[:, kb, h * 128:(h + 1) * 128],
                                        et[:, 0:w],
                                        start=(kb == 0),
                                        stop=(kb == kb_hi - 1))
                                if "O" not in variant:
                                    # adjacent col-tiled ones-matmuls: both
                                    # heads' denominators stream concurrently
                                    for ci, h in enumerate(hh):
                                        nc.tensor.matmul(
                                            pn[32 * ci:32 * ci + 1, off:512],
                                            ones_sb[:, 0:1],
                                            ets[h][:, 0:w],
                                            start=(kb == 0),
                                            stop=(kb == kb_hi - 1),
                                            tile_position=(0, 32 * ci),
                                            skip_group_check=True)
                            qsl = slice(qb * 512, (qb + 1) * 512)
                            # move head 1's denominator row (partition 32)
                            # next to head 0's so partition_broadcast (which
                            # reads via Q7 core 0, partitions 0-15) can serve
                            # both: PSUM->SBUF copy, SBUF row-move DMA.
                            pns = p_rb.tile([128, 512], F32, tag="pns")
                            nc.vector.tensor_copy(pns, pn[:, :])
                            rcb = p_rb.tile([1, 512], F32, tag="rcb")
                            nc.sync.dma_start(out=rcb[0:1, :],
                                              in_=pns[32:33, :])
                            for ci, h in enumerate(hh):
                                src = pns if ci == 0 else rcb
                                rc = p_rb.tile([1, 512], F32, tag="rc",
                                               name=f"rc{ci}")
                                nc.vector.reciprocal(rc, src[0:1, :])
                                rb = p_rb.tile([128, 512], F32, tag="rb")
                                nc.gpsimd.partition_broadcast(rb, rc[0:1, :])
                                nc.vector.tensor_mul(outT[:, h, qsl],
                                                     po[h], rb)

            if stages_done(variant, "c"):
                for h2 in range(HPC):
                    nc.sync.dma_start(
                        out=y[h2 * 64:(h2 + 1) * 64, 0:S],
                        in_=outT[:, h2, 0:S // 2].bitcast(F32))
            # ================= stage D: output projection =================
            run_d = not any(c in variant for c in "ACX")
            with tc.tile_pool(name="p_fin", bufs=3) as p_fin, \
                 tc.tile_pool(name="ps_d", bufs=4, space="PSUM") as ps_d:
                for sc in range(SC if run_d else 0):
                    fin = p_fin.tile([128, HIDDEN], BF, tag="fin")
                    for ob in range(HIDDEN // 512):
                        acc = ps_d.tile([128, 512], F32, tag="acc_d")
                        if "D" in fp8:
                            for ip in range(HPC // 2):
                                nc.tensor.matmul(
                                    acc,
                                    outT[:, 2 * ip:2 * ip + 2,
                                         sc * 128:(sc + 1) * 128],
                                    wo_sb[:, 2 * ip:2 * ip + 2,
                                          ob * 512:(ob + 1) * 512],
                                    start=(ip == 0), stop=(ip == HPC // 2 - 1),
                                    perf_mode=mybir.MatmulPerfMode.DoubleRow)
                        else:
                            for ic in range(HPC):
                                nc.tensor.matmul(
                                    acc,
                                    outT[:, ic, sc * 128:(sc + 1) * 128],
                                    wo_sb[:, ic, ob * 512:(ob + 1) * 512],
                                    start=(ic == 0), stop=(ic == HPC - 1))
                        osl = slice(ob * 512, (ob + 1) * 512)
                        if ob % 2 == 0:
                            nc.scalar.copy(fin[:, osl], acc)
                        else:
                            nc.vector.tensor_copy(fin[:, osl], acc)
                    nc.sync.dma_start(
                        out=y[sc * 128:(sc + 1) * 128, :], in_=fin)

    if finalize:
        nc.finalize()
    return nc


def stages_done(variant, st):
    return False


def _gather_lat(nc, latp, latg, dst_sb, variant):
    """AllGather one latent tensor across the TP group, then SBUF-load it.

    Variant "G" skips the collective (fills every quarter with the local
    shard) — wrong results, used only for timing ablation of the gather.
    """
    KC_L = LATENT // 128
    if "G" in variant:
        for m in range(KC_L):
            for t in range(TP):
                nc.sync.dma_start(
                    out=dst_sb[:, m, t * 512:(t + 1) * 512],
                    in_=latp[:, m, :])
        return
    nc.gpsimd.collective_compute(
        "AllGather", mybir.AluOpType.bypass,
        replica_groups=[[0, 1, 2, 3], [4, 5, 6, 7]],
        ins=[latp[:, :, :]], outs=[latg[:, :, :, :]])
    lg = latg.rearrange("t p g s -> p g t s")
    for m in range(KC_L):
        nc.sync.dma_start(out=dst_sb[:, m, :], in_=lg[:, m, :, :])


# ---------------------------------------------------------------------------
# host side

def _bf(a):
    import ml_dtypes
    return np.asarray(a, dtype=np.float32).astype(ml_dtypes.bfloat16)


def _f8(a):
    dt = mybir.dt.np(mybir.dt.float8e4)
    return np.clip(np.asarray(a, dtype=np.float32), -240, 240).astype(dt)


def host_inputs(x, Wq_d, Wkv_d, Wq_u, Wk_u, Wv_u, Wo, S=S_FULL, fp8=""):
    ca = _f8 if "A" in fp8 else _bf
    cd = _f8 if "D" in fp8 else _bf
    x = np.asarray(x, dtype=np.float32)

    inv_freq = 1.0 / (THETA ** (np.arange(0, HEAD_DIM, 2, dtype=np.float64)
                                / HEAD_DIM))
    pos = np.arange(S, dtype=np.float64)
    ang = pos[None, :] * np.concatenate([inv_freq, inv_freq])[:, None]
    COS = np.cos(ang)
    TAN = (np.sin(ang) / COS).astype(np.float32)
    COS = COS.astype(np.float32)

    MT = np.zeros((2, 128, 256), dtype=np.float32)
    r = np.arange(128)[:, None]
    c = np.arange(256)[None, :]
    MT[0] = np.where(c >= r, 0.0, NEG)
    MT[1] = np.where(c >= 128 + r, 0.0, NEG)

    PERM = np.zeros((128, 128), dtype=np.float32)
    for m in range(64):
        PERM[m + 64, m] = -1.0
        PERM[m, m + 64] = 1.0

    SQ = S // TP
    in_maps = []
    for core in range(N_CORES):
        b, tp = core // TP, core % TP
        sl = slice(tp * DSL, (tp + 1) * DSL)
        in_maps.append({
            "xT": ca(np.ascontiguousarray(
                x[b, :S].T[:, tp * SQ:(tp + 1) * SQ])),
            "wqd": ca(Wq_d),
            "wkvd": ca(Wkv_d),
            "wqu": _bf(np.ascontiguousarray(np.asarray(Wq_u)[:, sl])),
            "wku": _bf(np.ascontiguousarray(np.asarray(Wk_u)[:, sl])),
            "wvu": _bf(np.ascontiguousarray(np.asarray(Wv_u)[:, sl])),
            "wo": cd(np.ascontiguousarray(np.asarray(Wo)[sl, :])),
            "cosd": COS,
            "tand": TAN,
            "mtd": _bf(MT),
            "identd": _bf(np.eye(128, dtype=np.float32)),
            "permd": _bf(PERM),
            "onesd": _bf(np.ones((128, 1), dtype=np.float32)),
        })
    return in_maps


def assemble(results, S=S_FULL):
    out = np.zeros((B, S, HIDDEN), dtype=np.float32)
    for core in range(N_CORES):
        out[core // TP] += results[core]["y"].astype(np.float32)
    return out


_NC_CACHE = {}
FP8 = ""


def kernel(x, Wq_d, Wkv_d, Wq_u, Wk_u, Wv_u, Wo):
    S = x.shape[1]
    key = (S, FP8)
    if key not in _NC_CACHE:
        _NC_CACHE[key] = build_nc(S, fp8=FP8)
    nc = _NC_CACHE[key]
    in_maps = host_inputs(x, Wq_d, Wkv_d, Wq_u, Wk_u, Wv_u, Wo, S=S, fp8=FP8)
    res = run_bass_kernel_spmd(nc, in_maps, list(range(N_CORES)))
    return assemble(res.results, S=S)


# revision 3
# speedup vs baseline: 2.8075x; 1.3957x over previous
"""Trainium2 Bass kernel for MultiHeadLatentAttention (v2.3).

Structural changes vs v1 (420us baseline):
 - Stage A (down-projections) sharded over the 4-core TP group by sequence
   (each core computes S/4=512 positions of both latents), then one AllGather
   (bf16, 1MB/rank) rebuilds the full latents.  Saves ~3/4 of the largest
   GEMM's replicated PE time; the collective runs on TOPSP/SDMA silicon.
 - bf16 everywhere on the data path (same PE rate as f32r, 2x less DMA/SBUF,
   faster DVE), fp32 accumulation in PSUM throughout.
 - RoPE via the tan identity: rope(pa) = cos o (pa + P @ (pa o tan)), with the
   signed-permutation matmul ACCUMULATED onto the up-projection's PSUM bank.
   2 DVE ops per 512-block instead of 4.
 - Causal masks added on the PE (identity-stationary matmul accumulate into
   the score bank) instead of DVE tensor_adds.
 - exp(scores) stays on ACT (the only engine with a LUT); its ~60-80us is the
   attention-phase floor alongside the PE.
 - Attention runs in 2 passes of 2 heads so PSUM fits:
   2 po + 2 pn + 4 score bufs = 8 banks.
Layout is fully transposed ([feature, seq]) as v1: no PE transposes anywhere.
"""

import sys
from contextlib import ExitStack

sys.path.insert(0, "/opt/trn_rl_repo")

import numpy as np

import concourse.bass as bass
import concourse.mybir as mybir
import concourse.tile as tile
from concourse import bacc
from concourse.bass_utils import run_bass_kernel_spmd

HIDDEN = 2048
LATENT = 512
NUM_HEADS = 16
HEAD_DIM = 128
THETA = 10000.0
B = 2
S_FULL = 2048
N_CORES = 8
TP = 4
HPC = NUM_HEADS // TP          # 4 heads per core
DSL = HPC * HEAD_DIM           # 512: per-core slice of head dims

F32 = mybir.dt.float32
BF = mybir.dt.bfloat16
F8 = mybir.dt.float8e4

NEG = -1.0e30
SCALE = 1.0 / np.sqrt(HEAD_DIM)


def build_nc(S=S_FULL, finalize=True, iters=1, variant="", fp8=""):
    nc = bacc.Bacc(None, target_bir_lowering=False)

    KC_H = HIDDEN // 128       # 16
    KC_L = LATENT // 128       # 4
    NB = S // 512              # 4
    SC = S // 128              # 16
    SQ = S // TP               # 512 local seq positions for stage A

    DT_A = F8 if "A" in fp8 else BF
    DT_D = F8 if "D" in fp8 else BF
    xT = nc.dram_tensor("xT", [HIDDEN, SQ], DT_A, kind="ExternalInput")
    wqd = nc.dram_tensor("wqd", [HIDDEN, LATENT], DT_A, kind="ExternalInput")
    wkvd = nc.dram_tensor("wkvd", [HIDDEN, LATENT], DT_A, kind="ExternalInput")
    wqu = nc.dram_tensor("wqu", [LATENT, DSL], BF, kind="ExternalInput")
    wku = nc.dram_tensor("wku", [LATENT, DSL], BF, kind="ExternalInput")
    wvu = nc.dram_tensor("wvu", [LATENT, DSL], BF, kind="ExternalInput")
    wo = nc.dram_tensor("wo", [DSL, HIDDEN], DT_D, kind="ExternalInput")
    cosd = nc.dram_tensor("cosd", [128, S], F32, kind="ExternalInput")
    tand = nc.dram_tensor("tand", [128, S], F32, kind="ExternalInput")
    mtd = nc.dram_tensor("mtd", [2, 128, 256], BF, kind="ExternalInput")
    identd = nc.dram_tensor("identd", [128, 128], BF, kind="ExternalInput")
    permd = nc.dram_tensor("permd", [128, 128], BF, kind="ExternalInput")
    onesd = nc.dram_tensor("onesd", [128, 1], BF, kind="ExternalInput")
    latp = [nc.dram_tensor("latpq", [128, KC_L, SQ], BF),
            nc.dram_tensor("latpkv", [128, KC_L, SQ], BF)]
    latg = [nc.dram_tensor("latgq", [TP, 128, KC_L, SQ], BF),
            nc.dram_tensor("latgkv", [TP, 128, KC_L, SQ], BF)]
    y = nc.dram_tensor("y", [S, HIDDEN], BF, kind="ExternalOutput")

    unroll = 1
    if iters < 0:              # unrolled repeats (collective-safe timing)
        unroll, iters = -iters, 1
    with tile.TileContext(nc) as tc, ExitStack() as _es:
        if iters > 1:
            raise ValueError("use negative iters (unroll); collective "
                             "cannot sit inside For_i")
        for _u in range(unroll):
          with tc.tile_pool(name="p_out", bufs=1) as p_out, \
               tc.tile_pool(name="p_lat", bufs=1) as p_lat, \
               tc.tile_pool(name="p_tab", bufs=1) as p_tab, \
               tc.tile_pool(name="p_wpre", bufs=1) as p_wpre, \
               tc.tile_pool(name="p_const", bufs=1) as p_const:

            outT = p_out.tile([128, HPC, S], DT_D)
            latq = p_lat.tile([128, KC_L, S], BF)
            latkv = p_lat.tile([128, KC_L, S], BF)
            cos_sb = p_tab.tile([128, S], F32)
            tan_sb = p_tab.tile([128, S], F32)
            wqku_sb = p_wpre.tile([128, KC_L, 2, DSL], BF)
            wvu_sb = p_wpre.tile([128, KC_L, DSL], BF)
            wo_sb = p_wpre.tile([128, HPC, HIDDEN], DT_D)
            mask_sb = p_const.tile([128, 2, 256], BF)
            ones_sb = p_const.tile([128, 1], BF)
            perm_sb = p_const.tile([128, 128], BF)
            ident_sb = p_const.tile([128, 128], BF)

            # ================= stage A: sharded down projections ==========
            with tc.tile_pool(name="p_wd", bufs=1) as p_wd, \
                 tc.tile_pool(name="p_xt", bufs=1) as p_xt, \
                 tc.tile_pool(name="p_st", bufs=4) as p_st, \
                 tc.tile_pool(name="ps_a", bufs=4, space="PSUM") as ps_a:
                wqd_sb = p_wd.tile([128, KC_H, LATENT], DT_A)
                wkvd_sb = p_wd.tile([128, KC_H, LATENT], DT_A)
                x_sb = p_xt.tile([128, KC_H, SQ], DT_A)

                def load_w_col(w_sb, w_dram, m):
                    nc.sync.dma_start(
                        out=w_sb[:, :, m * 128:(m + 1) * 128],
                        in_=w_dram.rearrange("(kc p) l -> p kc l", p=128)
                        [:, :, m * 128:(m + 1) * 128])

                # prefetch: wqd m=0, x, remaining wd cols, then the rest of
                # the kernel's weights/tables (they overlap stage A compute).
                load_w_col(wqd_sb, wqd, 0)
                xr = xT.rearrange("(kc p) s -> p kc s", p=128)
                for kq in range(4):
                    nc.sync.dma_start(
                        out=x_sb[:, 4 * kq:4 * kq + 4, :],
                        in_=xr[:, 4 * kq:4 * kq + 4, :])
                load_w_col(wkvd_sb, wkvd, 0)
                for m in range(1, KC_L):
                    load_w_col(wqd_sb, wqd, m)
                    load_w_col(wkvd_sb, wkvd, m)
                nc.sync.dma_start(out=mask_sb, in_=mtd.rearrange("j p c -> p j c"))
                nc.sync.dma_start(out=ones_sb, in_=onesd[:, :])
                nc.sync.dma_start(out=perm_sb, in_=permd[:, :])
                nc.sync.dma_start(out=ident_sb, in_=identd[:, :])
                nc.sync.dma_start(out=cos_sb, in_=cosd[:, :])
                nc.sync.dma_start(out=tan_sb, in_=tand[:, :])
                nc.sync.dma_start(
                    out=wqku_sb[:, :, 0, :],
                    in_=wqu.rearrange("(kc p) d -> p kc d", p=128))
                nc.sync.dma_start(
                    out=wqku_sb[:, :, 1, :],
                    in_=wku.rearrange("(kc p) d -> p kc d", p=128))
                nc.sync.dma_start(
                    out=wvu_sb, in_=wvu.rearrange("(kc p) d -> p kc d", p=128))
                nc.sync.dma_start(
                    out=wo_sb, in_=wo.rearrange("(ic p) o -> p ic o", p=128))

                # preload the exp table set while stage A computes
                warm = p_st.tile([1, 1], BF, tag="warm")
                nc.scalar.activation(
                    out=warm, in_=ones_sb[0:1, 0:1],
                    func=mybir.ActivationFunctionType.Exp, scale=1.0)

                if "X" not in variant:
                    # q latent groups first, gather them while the kv groups
                    # compute; then the kv gather hides under q up-proj.
                    for g in range(2 * KC_L):
                        w_sb = wqd_sb if g < KC_L else wkvd_sb
                        m = g % KC_L
                        acc = ps_a.tile([128, SQ], F32, tag="acc_a")
                        if "A" in fp8:
                            for kp in range(KC_H // 2):
                                nc.tensor.matmul(
                                    acc,
                                    w_sb[:, 2 * kp:2 * kp + 2,
                                         m * 128:(m + 1) * 128],
                                    x_sb[:, 2 * kp:2 * kp + 2, :],
                                    start=(kp == 0),
                                    stop=(kp == KC_H // 2 - 1),
                                    perf_mode=mybir.MatmulPerfMode.DoubleRow)
                        else:
                            for kc in range(KC_H):
                                nc.tensor.matmul(
                                    acc,
                                    w_sb[:, kc, m * 128:(m + 1) * 128],
                                    x_sb[:, kc, :],
                                    start=(kc == 0), stop=(kc == KC_H - 1))
                        st = p_st.tile([128, SQ], BF, tag="st")
                        nc.vector.tensor_copy(st, acc)
                        nc.sync.dma_start(
                            out=latp[g // KC_L][:, g % KC_L, :], in_=st)
                        if g == KC_L - 1:
                            _gather_lat(nc, latp[0], latg[0], latq, variant)
                    _gather_lat(nc, latp[1], latg[1], latkv, variant)
                    # keep the PE's HAM clock un-throttled across the
                    # q-gather wait so the up-projections start at 2.4 GHz
                    dacc = ps_a.tile([128, 128], F32, tag="dacc")
                    for _d in range(32):
                        nc.tensor.matmul(
                            dacc, wqd_sb[:, 0, 0:128], x_sb[:, 0, 0:128],
                            start=True, stop=True)

            if stages_done(variant, "a"):
                nc.sync.dma_start(out=y[0:128, 0:S],
                                  in_=latq[:, 0, :].bitcast(F32))
                nc.sync.dma_start(out=y[128:256, 0:S],
                                  in_=latkv[:, 0, :].bitcast(F32))

            run_b = "A" not in variant
            # ========== stages B0/B: v + up-proj/rope, order chosen so the
            # kv AllGather hides under the q up-projections ================
            with tc.tile_pool(name="p_v", bufs=1) as p_v:
                v_sb = p_v.tile([128, SC, DSL], BF)

                # ============= stage B: up-proj + rope, all heads =========
                with tc.tile_pool(name="p_qk", bufs=1) as p_qk:
                    qkT = p_qk.tile([128, HPC, 2, S], BF)
                    with tc.tile_pool(name="p_rt", bufs=3) as p_rt, \
                         tc.tile_pool(name="ps_v", bufs=3,
                                      space="PSUM") as ps_v, \
                         tc.tile_pool(name="ps_b", bufs=2,
                                      space="PSUM") as ps_b:

                        def up_rope(h, di):
                            # two 512-blocks share a 2-bank psum tile so the
                            # rope DVE ops run 1024 wide (amortized overhead)
                            lat = latq if di == 0 else latkv
                            for np2 in range(NB // 2):
                                sl = slice(np2 * 1024, (np2 + 1) * 1024)
                                pa = ps_b.tile([128, 2, 512], F32, tag="pa")
                                for half in range(2):
                                    hs = slice((2 * np2 + half) * 512,
                                               (2 * np2 + half + 1) * 512)
                                    for kc in range(KC_L):
                                        nc.tensor.matmul(
                                            pa[:, half, :],
                                            wqku_sb[:, kc, di,
                                                    h * 128:(h + 1) * 128],
                                            lat[:, kc, hs],
                                            start=(kc == 0),
                                            stop=(kc == KC_L - 1))
                                t = p_rt.tile([128, 2, 512], BF, tag="t")
                                nc.vector.tensor_mul(t, pa, tan_sb[:, sl])
                                for half in range(2):
                                    nc.tensor.matmul(pa[:, half, :], perm_sb,
                                                     t[:, half, :],
                                                     start=False, stop=True,
                                                     skip_group_check=True)
                                nc.vector.tensor_mul(qkT[:, h, di, sl],
                                                     pa, cos_sb[:, sl])

                        def v_proj(sc):
                            acc = ps_v.tile([128, DSL], F32, tag="acc_v")
                            for kc in range(KC_L):
                                nc.tensor.matmul(
                                    acc,
                                    latkv[:, kc, sc * 128:(sc + 1) * 128],
                                    wvu_sb[:, kc, :],
                                    start=(kc == 0), stop=(kc == KC_L - 1))
                            nc.vector.tensor_copy(v_sb[:, sc, :], acc)

                        if run_b:
                            for h in range(HPC):
                                up_rope(h, 0)          # q (hides kv gather)
                            up_rope(0, 1)
                            up_rope(1, 1)              # k for pass 0
                            for sc in range(SC):
                                v_proj(sc)
                            up_rope(2, 1)
                            up_rope(3, 1)              # k for pass 1

                    # ============= stage C: attention, 2 heads per pass ===
                    with tc.tile_pool(name="p_at", bufs=6) as p_at, \
                         tc.tile_pool(name="p_rb", bufs=4) as p_rb:
                      for pp in range(2 if run_b else 0):
                        hh = (2 * pp, 2 * pp + 1)
                        with tc.tile_pool(name="ps_s", bufs=3,
                                          space="PSUM") as ps_s, \
                             tc.tile_pool(name="ps_o", bufs=4,
                                          space="PSUM") as ps_o, \
                             tc.tile_pool(name="ps_n", bufs=1,
                                          space="PSUM") as ps_n:
                          for qb in range(NB):
                            kb_hi = 4 * qb + 4
                            po = {}
                            for h in hh:
                                po[h] = ps_o.tile([128, 512], F32, tag="po",
                                                  name=f"po{h}")
                            # one bank holds both heads' denominators, at
                            # partitions 0 and 32 (col-groups 0/1 of the PE
                            # array so the two ones-matmuls run concurrently)
                            pn = ps_n.tile([128, 512], F32, tag="pn")
                            for kb in range(kb_hi):
                                j = kb - 4 * qb
                                off = j * 128 if j >= 0 else 0
                                w = 512 - off
                                q0 = qb * 512 + off
                                ets = {}
                                for h in hh:
                                    masked = (j >= 0 and "M" not in variant)
                                    ps = ps_s.tile([128, 512], F32, tag="ps")
                                    nc.tensor.matmul(
                                        ps[:, 0:w],
                                        qkT[:, h, 1,
                                            kb * 128:(kb + 1) * 128],
                                        qkT[:, h, 0, q0:q0 + w],
                                        start=True, stop=not masked)
                                    if masked:
                                        nc.tensor.matmul(
                                            ps[:, 0:128], ident_sb,
                                            mask_sb[:, 0, 0:128],
                                            start=False, stop=True,
                                            skip_group_check=True)
                                    et = p_at.tile([128, 512], BF, tag="et",
                                                   name=f"et{h}")
                                    if "E" in variant:
                                        nc.vector.tensor_copy(et[:, 0:w],
                                                              ps[:, 0:w])
                                    else:
                                        nc.scalar.activation(
                                            out=et[:, 0:w], in_=ps[:, 0:w],
                                            func=mybir.ActivationFunctionType.Exp,
                                            scale=float(SCALE))
                                    ets[h] = et
                                    nc.tensor.matmul(
                                        po[h][:, off:512],
                                        v_sb[:, kb, h * 128:(h + 1) * 128],
                                        et[:, 0:w],
                                        start=(kb == 0),
                                        stop=(kb == kb_hi - 1))
                                if "O" not in variant:
                                    # adjacent col-tiled ones-matmuls: both
                                    # heads' denominators stream concurrently
                                    for ci, h in enumerate(hh):
                                        nc.tensor.matmul(
                                            pn[32 * ci:32 * ci + 1, off:512],
                                            ones_sb[:, 0:1],
                                            ets[h][:, 0:w],
                                            start=(kb == 0),
                                            stop=(kb == kb_hi - 1),
                                            tile_position=(0, 32 * ci),
                                            skip_group_check=True)
                            qsl = slice(qb * 512, (qb + 1) * 512)
                            # move head 1's denominator row (partition 32)
                            # next to head 0's so partition_broadcast (which
                            # reads via Q7 core 0, partitions 0-15) can serve
                            # both: PSUM->SBUF copy, SBUF row-move DMA.
                            pns = p_rb.tile([128, 512], F32, tag="pns")
                            nc.vector.tensor_copy(pns, pn[:, :])
                            rcb = p_rb.tile([1, 512], F32, tag="rcb")
                            nc.sync.dma_start(out=rcb[0:1, :],
                                              in_=pns[32:33, :])
                            for ci, h in enumerate(hh):
                                src = pns if ci == 0 else rcb
                                rc = p_rb.tile([1, 512], F32, tag="rc",
                                               name=f"rc{ci}")
                                nc.vector.reciprocal(rc, src[0:1, :])
                                rb = p_rb.tile([128, 512], F32, tag="rb")
                                nc.gpsimd.partition_broadcast(rb, rc[0:1, :])
                                nc.vector.tensor_mul(outT[:, h, qsl],
                                                     po[h], rb)

            if stages_done(variant, "c"):
                for h2 in range(HPC):
                    nc.sync.dma_start(
                        out=y[h2 * 64:(h2 + 1) * 64, 0:S],
                        in_=outT[:, h2, 0:S // 2].bitcast(F32))
            # ================= stage D: output projection =================
            run_d = not any(c in variant for c in "ACX")
            with tc.tile_pool(name="p_fin", bufs=3) as p_fin, \
                 tc.tile_pool(name="ps_d", bufs=4, space="PSUM") as ps_d:
                for sc in range(SC if run_d else 0):
                    fin = p_fin.tile([128, HIDDEN], BF, tag="fin")
                    for ob in range(HIDDEN // 512):
                        acc = ps_d.tile([128, 512], F32, tag="acc_d")
                        if "D" in fp8:
                            for ip in range(HPC // 2):
                                nc.tensor.matmul(
                                    acc,
                                    outT[:, 2 * ip:2 * ip + 2,
                                         sc * 128:(sc + 1) * 128],
                                    wo_sb[:, 2 * ip:2 * ip + 2,
                                          ob * 512:(ob + 1) * 512],
                                    start=(ip == 0), stop=(ip == HPC // 2 - 1),
                                    perf_mode=mybir.MatmulPerfMode.DoubleRow)
                        else:
                            for ic in range(HPC):
                                nc.tensor.matmul(
                                    acc,
                                    outT[:, ic, sc * 128:(sc + 1) * 128],
                                    wo_sb[:, ic, ob * 512:(ob + 1) * 512],
                                    start=(ic == 0), stop=(ic == HPC - 1))
                        osl = slice(ob * 512, (ob + 1) * 512)
                        if ob % 2 == 0:
                            nc.scalar.copy(fin[:, osl], acc)
                        else:
                            nc.vector.tensor_copy(fin[:, osl], acc)
                    nc.sync.dma_start(
                        out=y[sc * 128:(sc + 1) * 128, :], in_=fin)

    if finalize:
        nc.finalize()
    return nc


def stages_done(variant, st):
    return False


def _gather_lat(nc, latp, latg, dst_sb, variant):
    """AllGather one latent tensor across the TP group, then SBUF-load it.

    Variant "G" skips the collective (fills every quarter with the local
    shard) — wrong results, used only for timing ablation of the gather.
    """
    KC_L = LATENT // 128
    if "G" in variant:
        for m in range(KC_L):
            for t in range(TP):
                nc.sync.dma_start(
                    out=dst_sb[:, m, t * 512:(t + 1) * 512],
                    in_=latp[:, m, :])
        return
    nc.gpsimd.collective_compute(
        "AllGather", mybir.AluOpType.bypass,
        replica_groups=[[0, 1, 2, 3], [4, 5, 6, 7]],
        ins=[latp[:, :, :]], outs=[latg[:, :, :, :]])
    lg = latg.rearrange("t p g s -> p g t s")
    for m in range(KC_L):
        nc.sync.dma_start(out=dst_sb[:, m, :], in_=lg[:, m, :, :])


# ---------------------------------------------------------------------------
# host side

def _bf(a):
    import ml_dtypes
    return np.asarray(a, dtype=np.float32).astype(ml_dtypes.bfloat16)


def _f8(a):
    dt = mybir.dt.np(mybir.dt.float8e4)
    return np.clip(np.asarray(a, dtype=np.float32), -240, 240).astype(dt)


def host_inputs(x, Wq_d, Wkv_d, Wq_u, Wk_u, Wv_u, Wo, S=S_FULL, fp8=""):
    ca = _f8 if "A" in fp8 else _bf
    cd = _f8 if "D" in fp8 else _bf
    x = np.asarray(x, dtype=np.float32)

    inv_freq = 1.0 / (THETA ** (np.arange(0, HEAD_DIM, 2, dtype=np.float64)
                                / HEAD_DIM))
    pos = np.arange(S, dtype=np.float64)
    ang = pos[None, :] * np.concatenate([inv_freq, inv_freq])[:, None]
    COS = np.cos(ang)
    TAN = (np.sin(ang) / COS).astype(np.float32)
    COS = COS.astype(np.float32)

    MT = np.zeros((2, 128, 256), dtype=np.float32)
    r = np.arange(128)[:, None]
    c = np.arange(256)[None, :]
    MT[0] = np.where(c >= r, 0.0, NEG)
    MT[1] = np.where(c >= 128 + r, 0.0, NEG)

    PERM = np.zeros((128, 128), dtype=np.float32)
    for m in range(64):
        PERM[m + 64, m] = -1.0
        PERM[m, m + 64] = 1.0

    SQ = S // TP
    in_maps = []
    for core in range(N_CORES):
        b, tp = core // TP, core % TP
        sl = slice(tp * DSL, (tp + 1) * DSL)
        in_maps.append({
            "xT": ca(np.ascontiguousarray(
                x[b, :S].T[:, tp * SQ:(tp + 1) * SQ])),
            "wqd": ca(Wq_d),
            "wkvd": ca(Wkv_d),
            "wqu": _bf(np.ascontiguousarray(np.asarray(Wq_u)[:, sl])),
            "wku": _bf(np.ascontiguousarray(np.asarray(Wk_u)[:, sl])),
            "wvu": _bf(np.ascontiguousarray(np.asarray(Wv_u)[:, sl])),
            "wo": cd(np.ascontiguousarray(np.asarray(Wo)[sl, :])),
            "cosd": COS,
            "tand": TAN,
            "mtd": _bf(MT),
            "identd": _bf(np.eye(128, dtype=np.float32)),
            "permd": _bf(PERM),
            "onesd": _bf(np.ones((128, 1), dtype=np.float32)),
        })
    return in_maps


def assemble(results, S=S_FULL):
    out = np.zeros((B, S, HIDDEN), dtype=np.float32)
    for core in range(N_CORES):
        out[core // TP] += results[core]["y"].astype(np.float32)
    return out


_NC_CACHE = {}
FP8 = ""


def kernel(x, Wq_d, Wkv_d, Wq_u, Wk_u, Wv_u, Wo):
    S = x.shape[1]
    key = (S, FP8)
    if key not in _NC_CACHE:
        _NC_CACHE[key] = build_nc(S, fp8=FP8)
    nc = _NC_CACHE[key]
    in_maps = host_inputs(x, Wq_d, Wkv_d, Wq_u, Wk_u, Wv_u, Wo, S=S, fp8=FP8)
    res = run_bass_kernel_spmd(nc, in_maps, list(range(N_CORES)))
    return assemble(res.results, S=S)
